# revision 37
# baseline (speedup 1.0000x reference)
"""Trainium2 Bass kernel for DifferentiableRGBtoVel (soft-nearest-neighbor
colormap inversion).

velocity(p) = sum_k v_k e^{-100 d_k(p)} / sum_k e^{-100 d_k(p)},
d_k(p) = |p - c_k|^2.

Softmax stabilizer: the linear surrogate B_p = 100*sum_c(p_c) - 37.5 of
100|p|^2 (minimax linear fit of x^2 on [0,1]) keeps every exponent inside
fp32 range; the shift cancels exactly in the num/den ratio.

All matmuls are BF16 (the PE's native 1-cycle/row path with fast weight
load; fp32 runs at 4 cycles/row and float32r measures ~2 cycles/row on this
silicon AND keeps the HAM clock-gate cold).  Precision is recovered with
multi-term Dekker splits in the CONTRACTION dimension, which is free (cost =
streamed columns only): w = w1+w2+w3 and p = p1+p2+p3 (bf16 each), bias =
b1+b2+b3, keeping products (w1,p1..p3),(w2,p1..p2),(w3,p1),(bias,1) -> 21
contraction rows per K-half.  bf16 x bf16 products are exact in the fp32
PSUM accumulate, so scores are accurate to ~1e-5; only the exp table and
v_i table carry bf16 rounding (~4e-3 output error vs the 2e-2 gate).

Per-core pipeline in [k, pix] layout (partition = colormap index), tiles of
512 pixels ([128, 1024] PSUM = [A|B], 4-deep for scheduling slack):
  scores: one contraction-21 bf16 matmul per 128-color half; halves run
          concurrently in PE row groups 0/32 (A rows 0-20, B rows 32-52).
  exp:    ONE ACT instruction per tile (FD=1024), func=Exp, scale=200,
          bf16 output.
  num/den: bf16 [128,2] matmuls (cols = [1, v_k]); output [2,512] at PSUM
          partitions 0-1 in the tile's own consumed score region.
  divide: DVE copy [2,512] -> a [2, 16*512] staging tile; per 16 tiles TWO
          partition-slice DMAs redistribute dens -> dn rows j..j+15 and
          nums -> dn rows 64+j.. (plain slices only: a rearranged multi-
          partition-dim view mis-lowers its offset and clobbers low SBUF;
          2 DMAs/batch also keeps the division's queue-wait list under the
          7-semaphore instruction cap).  DVE reciprocal + multiply per
          64-tile group, one output DMA per group.
"""

import numpy as np
import ml_dtypes

import concourse.bass as bass
import concourse.mybir as mybir
import concourse.tile as tile_mod
from concourse.tile import TileContext
from concourse.vector_clock import ScopedClock, VectorClock
from concourse.bass_utils import run_bass_kernel_spmd

# ---------------------------------------------------------------- constants
N_CORES = 8
NB, C, H, W = 4, 3, 512, 512
K = 256
KH = 128
PIX_PER_CORE = NB * H * W // N_CORES   # 131072
TILE_PIX = 512                 # pixels per tile
GROUP = 64                     # tiles per division group
IMG_BATCH = 8                  # tiles per image DMA
NROW = 21                      # contraction rows per K-half

_FP32 = mybir.dt.float32
_BF16 = mybir.dt.bfloat16
_BF = ml_dtypes.bfloat16


# ------------------------------------------------- walrus sync-wait limits
# This walrus build rejects instructions carrying more than one sem wait
# ("Too many sync wait commands"); split extras onto same-engine NoOps.
def _split_drain_and_barrier(self, tick_clock, wait_clock):
    nc = self.nc
    vec = list(tick_clock.global_clock)
    for i, v in enumerate(vec):
        if v > 0:
            w = [0] * len(vec)
            w[i] = v
            inst = nc.sync.nop(nofuse=True, hint="split_drain_wait")
            wait_clock.add_sem_waits(inst.ins, ScopedClock({None: VectorClock(w)}))
    nc.sync.drain()
    nc.all_engine_barrier()
    assert self.sems is not None
    popped = nc._tile_sem_poison_stack.pop()
    assert popped is self._sem_poison
    nc.clear_and_free_semaphores(list(self.sems.allocated().values()))
    nc.all_engine_barrier()


tile_mod.TileContext._drain_and_barrier = _split_drain_and_barrier

MAX_WAITS = 1


def _split_excess_waits(nc, maxw=MAX_WAITS):
    for f in nc.m.functions:
        for bb in f.blocks:
            out = []
            for inst in bb.instructions:
                si = inst.sync_info
                if si is not None and len(si.on_wait) > maxw:
                    waits = list(si.on_wait)
                    excess, keep = waits[:-maxw], waits[-maxw:]
                    for i in range(0, len(excess), maxw):
                        nop = mybir.InstNoOp(
                            name=nc.get_next_instruction_name(),
                            sync_info=mybir.SyncInfo(
                                on_wait=excess[i:i + maxw], on_update=[]),
                            bass_nofuse=True,
                            engine=inst.engine,
                        )
                        out.append(nop)
                    inst.sync_info = mybir.SyncInfo(
                        on_wait=keep, on_update=list(si.on_update))
                out.append(inst)
            bb.instructions = out


# ------------------------------------------------------------- bass builder
def build_kernel(pix_per_core: int = PIX_PER_CORE, split_waits: bool = True):
    n_tiles = pix_per_core // TILE_PIX
    n_groups = (n_tiles + GROUP - 1) // GROUP

    nc = bass.Bass(trn_type="TRN2", name="rgb2vel")
    imgD = nc.dram_tensor("img", [2 * NROW, pix_per_core], _BF16,
                          kind="ExternalInput")
    cmD = nc.dram_tensor("cmt", [NROW, K], _BF16, kind="ExternalInput")
    vmD = nc.dram_tensor("vmat", [KH, 4], _BF16, kind="ExternalInput")
    velD = nc.dram_tensor("vel", [pix_per_core // 512, 512], _FP32,
                          kind="ExternalOutput")

    ExpF = mybir.ActivationFunctionType.Exp

    with TileContext(nc) as tc:
        # HAM heater: ~5us of back-to-back matmuls at kernel start flips the
        # PE clock-gate to K=8/8 (2.4 GHz); the ~90%-busy steady state then
        # keeps it warm.  Transient pools, released before the main pools.
        with (
            tc.tile_pool(name="heat", bufs=1) as hpool,
            tc.tile_pool(name="heatp", bufs=1, space="PSUM") as hppool,
        ):
            hs = hpool.tile([KH, TILE_PIX], _BF16, tag="hs")
            nc.vector.memset(hs[:], 0.0)
            hw = hpool.tile([KH, 2], _BF16, tag="hw")
            nc.vector.memset(hw[:], 0.0)
            hp = hppool.tile([2, TILE_PIX], _FP32, tag="hp")
            for _ in range(24):
                nc.tensor.matmul(hp[:], lhsT=hw[:], rhs=hs[:],
                                 start=True, stop=True)
        with (
            tc.tile_pool(name="const", bufs=1) as cpool,
            tc.tile_pool(name="img", bufs=3) as ipool,
            tc.tile_pool(name="exp", bufs=6) as epool,
            tc.tile_pool(name="stg", bufs=2) as stgpool,
            tc.tile_pool(name="acc", bufs=4) as accpool,
            tc.tile_pool(name="divp", bufs=4) as dpool,
            tc.tile_pool(name="score", bufs=4, space="PSUM") as spool,
        ):
            # persistent constants: cm rows 0-20 = half A, 32-52 = half B
            cm = cpool.tile([32 + NROW, KH], _BF16, tag="cm")
            nc.sync.dma_start(cm[0:NROW, :], cmD[:, 0:KH])
            nc.sync.dma_start(cm[32:32 + NROW, :], cmD[:, KH:K])
            vm = cpool.tile([KH, 4], _BF16, tag="vm")
            nc.sync.dma_start(vm[:], vmD[:])
            cmA = cm[0:NROW, :]
            cmB = cm[32:32 + NROW, :]
            vmr = vm[:]

            state = {"img": None, "pending": [], "stg": None, "stg_n": 0,
                     "stg_j0": 0}
            STG_TILES = 16  # tiles per scatter DMA (keeps the division's
                            # DMA-queue wait list under the 7-sem HW cap)

            def emit_tail():
                # V chain + evacuation for the oldest pending tile.
                t, j, ps, ex, dn = state["pending"].pop(0)
                out = ps[0:2, 0:TILE_PIX]
                nc.tensor.matmul(out, lhsT=vmr[:, 0:2],
                                 rhs=ex[:, 0:TILE_PIX],
                                 start=True, stop=False)
                nc.tensor.matmul(out, lhsT=vmr[:, 2:4],
                                 rhs=ex[:, TILE_PIX:2 * TILE_PIX],
                                 start=False, stop=True)
                if state["stg"] is None:
                    stg_t = stgpool.tile(
                        [2, STG_TILES * TILE_PIX], _FP32, tag="stg")
                    state["stg"] = stg_t
                    state["stg_n"] = 0
                    state["stg_j0"] = j
                stg = state["stg"]
                off = state["stg_n"] * TILE_PIX
                nc.vector.tensor_copy(stg[:, off:off + TILE_PIX], out)
                state["stg_n"] += 1
                if state["stg_n"] == STG_TILES:
                    j0b = state["stg_j0"]
                    nc.sync.dma_start(dn[j0b:j0b + STG_TILES, :], stg[0:1, :])
                    nc.sync.dma_start(dn[64 + j0b:64 + j0b + STG_TILES, :],
                                      stg[1:2, :])
                    state["stg"] = None

            def do_tile(t, dn, j):
                if t % IMG_BATCH == 0:
                    imgt = ipool.tile([32 + NROW, IMG_BATCH * TILE_PIX],
                                      _BF16, tag="img")
                    sl = slice(t * TILE_PIX, (t + IMG_BATCH) * TILE_PIX)
                    nc.sync.dma_start(imgt[0:NROW, :], imgD[0:NROW, sl])
                    nc.sync.dma_start(imgt[32:32 + NROW, :],
                                      imgD[NROW:2 * NROW, sl])
                    state["img"] = imgt
                img = state["img"]

                # Emit the oldest pending tile's V chain BEFORE allocating a
                # new PSUM tile: spool has 4 slots, so at most 4 tiles may be
                # alive or the scheduler is forced into a serial schedule.
                while len(state["pending"]) >= 3:
                    emit_tail()

                ioff = (t % IMG_BATCH) * TILE_PIX
                ra = img[0:NROW, ioff:ioff + TILE_PIX]
                rb = img[32:32 + NROW, ioff:ioff + TILE_PIX]
                ps = spool.tile([128, 2 * TILE_PIX], _FP32, tag="score")
                nc.tensor.matmul(ps[:, 0:TILE_PIX], lhsT=cmA,
                                 rhs=ra, start=True, stop=True)
                nc.tensor.matmul(ps[:, TILE_PIX:2 * TILE_PIX],
                                 lhsT=cmB, rhs=rb, start=True, stop=True)

                ex = epool.tile([128, 2 * TILE_PIX], _BF16, tag="exp")
                nc.scalar.activation(ex[:], ps[:], ExpF, bias=0.0, scale=200.0)
                state["pending"].append((t, j, ps, ex, dn))

            def do_group(g, gtiles):
                dn = accpool.tile([128, 512], _FP32, tag="dn")
                for j in range(gtiles):
                    do_tile(g * GROUP + j, dn, j)
                while state["pending"]:
                    emit_tail()
                rows = gtiles
                nsh = dpool.tile([64, 512], _FP32, tag="nsh")
                nc.sync.dma_start(nsh[0:rows, :], dn[64:64 + rows, :])
                rcp = dpool.tile([64, 512], _FP32, tag="rcp")
                nc.vector.reciprocal(rcp[0:rows, :], dn[0:rows, :])
                vel = dpool.tile([64, 512], _FP32, tag="vel")
                nc.vector.tensor_tensor(
                    vel[0:rows, :], nsh[0:rows, :], rcp[0:rows, :],
                    mybir.AluOpType.mult)
                nc.sync.dma_start(velD[g * GROUP:g * GROUP + rows, :],
                                  vel[0:rows, :])

            for g in range(n_groups):
                do_group(g, min(GROUP, n_tiles - g * GROUP))

    if split_waits:
        _split_excess_waits(nc)
    return nc


# ----------------------------------------------------------- host wrapper
_CACHE = {}


def _get_nc(pix_per_core):
    if pix_per_core not in _CACHE:
        _CACHE[pix_per_core] = build_kernel(pix_per_core)
    return _CACHE[pix_per_core]


def _bf_splits(x, n):
    """n-term Dekker split of fp32 array into bf16 parts (sum == x to
    ~2^-8n relative)."""
    outs = []
    r = np.asarray(x, np.float32)
    for _ in range(n):
        b = r.astype(_BF)
        outs.append(b)
        r = np.float32(r - b.astype(np.float32))
    return outs


def _prep_consts(cmap, v_i):
    cmap = np.asarray(cmap, np.float64)
    v_i = np.asarray(v_i, np.float32)
    w = np.float32(cmap - 0.5)                  # [K,3]
    w1, w2, w3 = _bf_splits(w, 3)
    c2 = np.sum(cmap * cmap, axis=1)            # fp64
    b = (37.5 - 100.0 * c2) / 200.0             # fp64 [K]
    b1, b2, b3 = _bf_splits(np.float32(b), 3)
    # lhsT row i pairs with image row i:
    # 0-2 (w1,p1)  3-5 (w1,p2)  6-8 (w1,p3)  9-11 (w2,p1)  12-14 (w2,p2)
    # 15-17 (w3,p1)  18-20 (b1|b2|b3, ones)
    cmt = np.empty((NROW, K), _BF)
    cmt[0:3] = w1.T
    cmt[3:6] = w1.T
    cmt[6:9] = w1.T
    cmt[9:12] = w2.T
    cmt[12:15] = w2.T
    cmt[15:18] = w3.T
    cmt[18] = b1
    cmt[19] = b2
    cmt[20] = b3
    vmat = np.empty((KH, 4), _BF)
    vmat[:, 0] = 1.0
    vmat[:, 1] = v_i[0:KH].astype(_BF)
    vmat[:, 2] = 1.0
    vmat[:, 3] = v_i[KH:K].astype(_BF)
    return cmt, vmat


def _prep_image_slab(slab):
    """slab: [3, n] float32 -> [2*NROW, n] bf16 rows (A half then B copy)."""
    n = slab.shape[1]
    p1, p2, p3 = _bf_splits(slab, 3)
    img = np.empty((2 * NROW, n), _BF)
    img[0:3] = p1
    img[3:6] = p2
    img[6:9] = p3
    img[9:12] = p1
    img[12:15] = p2
    img[15:18] = p1
    img[18:21] = 1.0
    img[NROW:2 * NROW] = img[0:NROW]
    return img


def _kernel_impl(image, cmap, v_i, _trace=False):
    image = np.ascontiguousarray(np.asarray(image, np.float32))
    cmt, vmat = _prep_consts(cmap, v_i)

    rows_per_core = NB * H // N_CORES          # 256 rows of H per core
    in_maps = []
    for i in range(N_CORES):
        n = (i * rows_per_core) // H
        h0 = (i * rows_per_core) % H
        slab = image[n, :, h0:h0 + rows_per_core, :].reshape(3, -1)
        in_maps.append({"img": _prep_image_slab(slab), "cmt": cmt,
                        "vmat": vmat})

    nc = _get_nc(PIX_PER_CORE)
    res = run_bass_kernel_spmd(nc, in_maps, core_ids=list(range(N_CORES)),
                               trace=_trace)
    out = np.empty((NB, H, W), np.float32)
    for i in range(N_CORES):
        n = (i * rows_per_core) // H
        h0 = (i * rows_per_core) % H
        out[n, h0:h0 + rows_per_core, :] = \
            res.results[i]["vel"].reshape(rows_per_core, W)
    return out, res


def kernel(image, cmap, v_i):
    out, _ = _kernel_impl(image, cmap, v_i)
    return out


# revision 38
# speedup vs baseline: 1.0206x; 1.0206x over previous
"""Trainium2 Bass kernel for DifferentiableRGBtoVel (soft-nearest-neighbor
colormap inversion).

velocity(p) = sum_k v_k e^{-100 d_k(p)} / sum_k e^{-100 d_k(p)},
d_k(p) = |p - c_k|^2.

Softmax stabilizer: the linear surrogate B_p = 100*sum_c(p_c) - 37.5 of
100|p|^2 (minimax linear fit of x^2 on [0,1]) keeps every exponent inside
fp32 range; the shift cancels exactly in the num/den ratio.

All matmuls are BF16 (the PE's native 1-cycle/row path with fast weight
load; fp32 runs at 4 cycles/row and float32r measures ~2 cycles/row on this
silicon AND keeps the HAM clock-gate cold).  Precision is recovered with
multi-term Dekker splits in the CONTRACTION dimension, which is free (cost =
streamed columns only): w = w1+w2+w3 and p = p1+p2+p3 (bf16 each), bias =
b1+b2+b3, keeping products (w1,p1..p3),(w2,p1..p2),(w3,p1),(bias,1) -> 21
contraction rows per K-half.  bf16 x bf16 products are exact in the fp32
PSUM accumulate, so scores are accurate to ~1e-5; only the exp table and
v_i table carry bf16 rounding (~4e-3 output error vs the 2e-2 gate).

Per-core pipeline in [k, pix] layout (partition = colormap index), tiles of
512 pixels ([128, 1024] PSUM = [A|B], 4-deep for scheduling slack):
  scores: one contraction-21 bf16 matmul per 128-color half; halves run
          concurrently in PE row groups 0/32 (A rows 0-20, B rows 32-52).
  exp:    ONE ACT instruction per tile (FD=1024), func=Exp, scale=200,
          bf16 output.
  num/den: bf16 [128,2] matmuls (cols = [1, v_k]); output [2,512] at PSUM
          partitions 0-1 in the tile's own consumed score region.
  divide: DVE copy [2,512] -> a [2, 16*512] staging tile; per 16 tiles TWO
          partition-slice DMAs redistribute dens -> dn rows j..j+15 and
          nums -> dn rows 64+j.. (plain slices only: a rearranged multi-
          partition-dim view mis-lowers its offset and clobbers low SBUF;
          2 DMAs/batch also keeps the division's queue-wait list under the
          7-semaphore instruction cap).  DVE reciprocal + multiply per
          64-tile group, one output DMA per group.
"""

import numpy as np
import ml_dtypes

import concourse.bass as bass
import concourse.mybir as mybir
import concourse.tile as tile_mod
from concourse.tile import TileContext
from concourse.vector_clock import ScopedClock, VectorClock
from concourse.bass_utils import run_bass_kernel_spmd

# ---------------------------------------------------------------- constants
N_CORES = 8
NB, C, H, W = 4, 3, 512, 512
K = 256
KH = 128
PIX_PER_CORE = NB * H * W // N_CORES   # 131072
TILE_PIX = 512                 # pixels per tile
GROUP = 64                     # tiles per division group
IMG_BATCH = 8                  # tiles per image DMA
NROW = 21                      # contraction rows per K-half

_FP32 = mybir.dt.float32
_BF16 = mybir.dt.bfloat16
_BF = ml_dtypes.bfloat16


# ------------------------------------------------- walrus sync-wait limits
# This walrus build rejects instructions carrying more than one sem wait
# ("Too many sync wait commands"); split extras onto same-engine NoOps.
def _split_drain_and_barrier(self, tick_clock, wait_clock):
    nc = self.nc
    vec = list(tick_clock.global_clock)
    for i, v in enumerate(vec):
        if v > 0:
            w = [0] * len(vec)
            w[i] = v
            inst = nc.sync.nop(nofuse=True, hint="split_drain_wait")
            wait_clock.add_sem_waits(inst.ins, ScopedClock({None: VectorClock(w)}))
    nc.sync.drain()
    nc.all_engine_barrier()
    assert self.sems is not None
    popped = nc._tile_sem_poison_stack.pop()
    assert popped is self._sem_poison
    nc.clear_and_free_semaphores(list(self.sems.allocated().values()))
    nc.all_engine_barrier()


tile_mod.TileContext._drain_and_barrier = _split_drain_and_barrier

MAX_WAITS = 1


def _split_excess_waits(nc, maxw=MAX_WAITS):
    for f in nc.m.functions:
        for bb in f.blocks:
            out = []
            for inst in bb.instructions:
                si = inst.sync_info
                if si is not None and len(si.on_wait) > maxw:
                    waits = list(si.on_wait)
                    excess, keep = waits[:-maxw], waits[-maxw:]
                    for i in range(0, len(excess), maxw):
                        nop = mybir.InstNoOp(
                            name=nc.get_next_instruction_name(),
                            sync_info=mybir.SyncInfo(
                                on_wait=excess[i:i + maxw], on_update=[]),
                            bass_nofuse=True,
                            engine=inst.engine,
                        )
                        out.append(nop)
                    inst.sync_info = mybir.SyncInfo(
                        on_wait=keep, on_update=list(si.on_update))
                out.append(inst)
            bb.instructions = out


# ------------------------------------------------------------- bass builder
def build_kernel(pix_per_core: int = PIX_PER_CORE, split_waits: bool = True):
    n_tiles = pix_per_core // TILE_PIX
    n_groups = (n_tiles + GROUP - 1) // GROUP

    nc = bass.Bass(trn_type="TRN2", name="rgb2vel")
    imgD = nc.dram_tensor("img", [2 * NROW, pix_per_core], _BF16,
                          kind="ExternalInput")
    cmD = nc.dram_tensor("cmt", [NROW, K], _BF16, kind="ExternalInput")
    vmD = nc.dram_tensor("vmat", [KH, 4], _BF16, kind="ExternalInput")
    velD = nc.dram_tensor("vel", [pix_per_core // 512, 512], _FP32,
                          kind="ExternalOutput")

    ExpF = mybir.ActivationFunctionType.Exp

    with TileContext(nc) as tc:
        with (
            tc.tile_pool(name="const", bufs=1) as cpool,
            tc.tile_pool(name="img", bufs=3) as ipool,
            tc.tile_pool(name="exp", bufs=6) as epool,
            tc.tile_pool(name="stg", bufs=2) as stgpool,
            tc.tile_pool(name="acc", bufs=4) as accpool,
            tc.tile_pool(name="divp", bufs=4) as dpool,
            tc.tile_pool(name="score", bufs=4, space="PSUM") as spool,
        ):
            # persistent constants: cm rows 0-20 = half A, 32-52 = half B
            cm = cpool.tile([32 + NROW, KH], _BF16, tag="cm")
            nc.sync.dma_start(cm[0:NROW, :], cmD[:, 0:KH])
            nc.sync.dma_start(cm[32:32 + NROW, :], cmD[:, KH:K])
            vm = cpool.tile([KH, 4], _BF16, tag="vm")
            nc.sync.dma_start(vm[:], vmD[:])
            cmA = cm[0:NROW, :]
            cmB = cm[32:32 + NROW, :]
            vmr = vm[:]

            state = {"img": None, "pending": [], "stg": None, "stg_n": 0,
                     "stg_j0": 0}
            STG_TILES = 16  # tiles per scatter DMA (keeps the division's
                            # DMA-queue wait list under the 7-sem HW cap)

            def emit_tail():
                # V chain + evacuation for the oldest pending tile.
                t, j, ps, ex, dn = state["pending"].pop(0)
                out = ps[0:2, 0:TILE_PIX]
                nc.tensor.matmul(out, lhsT=vmr[:, 0:2],
                                 rhs=ex[:, 0:TILE_PIX],
                                 start=True, stop=False)
                nc.tensor.matmul(out, lhsT=vmr[:, 2:4],
                                 rhs=ex[:, TILE_PIX:2 * TILE_PIX],
                                 start=False, stop=True)
                if state["stg"] is None:
                    stg_t = stgpool.tile(
                        [2, STG_TILES * TILE_PIX], _FP32, tag="stg")
                    state["stg"] = stg_t
                    state["stg_n"] = 0
                    state["stg_j0"] = j
                stg = state["stg"]
                off = state["stg_n"] * TILE_PIX
                nc.vector.tensor_copy(stg[:, off:off + TILE_PIX], out)
                state["stg_n"] += 1
                if state["stg_n"] == STG_TILES:
                    j0b = state["stg_j0"]
                    nc.sync.dma_start(dn[j0b:j0b + STG_TILES, :], stg[0:1, :])
                    nc.sync.dma_start(dn[64 + j0b:64 + j0b + STG_TILES, :],
                                      stg[1:2, :])
                    state["stg"] = None

            def do_tile(t, dn, j):
                if t % IMG_BATCH == 0:
                    imgt = ipool.tile([32 + NROW, IMG_BATCH * TILE_PIX],
                                      _BF16, tag="img")
                    sl = slice(t * TILE_PIX, (t + IMG_BATCH) * TILE_PIX)
                    nc.sync.dma_start(imgt[0:NROW, :], imgD[0:NROW, sl])
                    nc.sync.dma_start(imgt[32:32 + NROW, :],
                                      imgD[NROW:2 * NROW, sl])
                    state["img"] = imgt
                img = state["img"]

                # Emit the oldest pending tile's V chain BEFORE allocating a
                # new PSUM tile: spool has 4 slots, so at most 4 tiles may be
                # alive or the scheduler is forced into a serial schedule.
                while len(state["pending"]) >= 3:
                    emit_tail()

                ioff = (t % IMG_BATCH) * TILE_PIX
                ra = img[0:NROW, ioff:ioff + TILE_PIX]
                rb = img[32:32 + NROW, ioff:ioff + TILE_PIX]
                ps = spool.tile([128, 2 * TILE_PIX], _FP32, tag="score")
                nc.tensor.matmul(ps[:, 0:TILE_PIX], lhsT=cmA,
                                 rhs=ra, start=True, stop=True)
                nc.tensor.matmul(ps[:, TILE_PIX:2 * TILE_PIX],
                                 lhsT=cmB, rhs=rb, start=True, stop=True)

                ex = epool.tile([128, 2 * TILE_PIX], _BF16, tag="exp")
                nc.scalar.activation(ex[:], ps[:], ExpF, bias=0.0, scale=200.0)
                state["pending"].append((t, j, ps, ex, dn))

            def do_group(g, gtiles):
                dn = accpool.tile([128, 512], _FP32, tag="dn")
                for j in range(gtiles):
                    do_tile(g * GROUP + j, dn, j)
                while state["pending"]:
                    emit_tail()
                rows = gtiles
                nsh = dpool.tile([64, 512], _FP32, tag="nsh")
                nc.sync.dma_start(nsh[0:rows, :], dn[64:64 + rows, :])
                rcp = dpool.tile([64, 512], _FP32, tag="rcp")
                nc.vector.reciprocal(rcp[0:rows, :], dn[0:rows, :])
                vel = dpool.tile([64, 512], _FP32, tag="vel")
                nc.vector.tensor_tensor(
                    vel[0:rows, :], nsh[0:rows, :], rcp[0:rows, :],
                    mybir.AluOpType.mult)
                nc.sync.dma_start(velD[g * GROUP:g * GROUP + rows, :],
                                  vel[0:rows, :])

            for g in range(n_groups):
                do_group(g, min(GROUP, n_tiles - g * GROUP))

    if split_waits:
        _split_excess_waits(nc)
    return nc


# ----------------------------------------------------------- host wrapper
_CACHE = {}


def _get_nc(pix_per_core):
    if pix_per_core not in _CACHE:
        _CACHE[pix_per_core] = build_kernel(pix_per_core)
    return _CACHE[pix_per_core]


def _bf_splits(x, n):
    """n-term Dekker split of fp32 array into bf16 parts (sum == x to
    ~2^-8n relative)."""
    outs = []
    r = np.asarray(x, np.float32)
    for _ in range(n):
        b = r.astype(_BF)
        outs.append(b)
        r = np.float32(r - b.astype(np.float32))
    return outs


def _prep_consts(cmap, v_i):
    cmap = np.asarray(cmap, np.float64)
    v_i = np.asarray(v_i, np.float32)
    w = np.float32(cmap - 0.5)                  # [K,3]
    w1, w2, w3 = _bf_splits(w, 3)
    c2 = np.sum(cmap * cmap, axis=1)            # fp64
    b = (37.5 - 100.0 * c2) / 200.0             # fp64 [K]
    b1, b2, b3 = _bf_splits(np.float32(b), 3)
    # lhsT row i pairs with image row i:
    # 0-2 (w1,p1)  3-5 (w1,p2)  6-8 (w1,p3)  9-11 (w2,p1)  12-14 (w2,p2)
    # 15-17 (w3,p1)  18-20 (b1|b2|b3, ones)
    cmt = np.empty((NROW, K), _BF)
    cmt[0:3] = w1.T
    cmt[3:6] = w1.T
    cmt[6:9] = w1.T
    cmt[9:12] = w2.T
    cmt[12:15] = w2.T
    cmt[15:18] = w3.T
    cmt[18] = b1
    cmt[19] = b2
    cmt[20] = b3
    vmat = np.empty((KH, 4), _BF)
    vmat[:, 0] = 1.0
    vmat[:, 1] = v_i[0:KH].astype(_BF)
    vmat[:, 2] = 1.0
    vmat[:, 3] = v_i[KH:K].astype(_BF)
    return cmt, vmat


def _prep_image_slab(slab):
    """slab: [3, n] float32 -> [2*NROW, n] bf16 rows (A half then B copy)."""
    n = slab.shape[1]
    p1, p2, p3 = _bf_splits(slab, 3)
    img = np.empty((2 * NROW, n), _BF)
    img[0:3] = p1
    img[3:6] = p2
    img[6:9] = p3
    img[9:12] = p1
    img[12:15] = p2
    img[15:18] = p1
    img[18:21] = 1.0
    img[NROW:2 * NROW] = img[0:NROW]
    return img


def _kernel_impl(image, cmap, v_i, _trace=False):
    image = np.ascontiguousarray(np.asarray(image, np.float32))
    cmt, vmat = _prep_consts(cmap, v_i)

    rows_per_core = NB * H // N_CORES          # 256 rows of H per core
    in_maps = []
    for i in range(N_CORES):
        n = (i * rows_per_core) // H
        h0 = (i * rows_per_core) % H
        slab = image[n, :, h0:h0 + rows_per_core, :].reshape(3, -1)
        in_maps.append({"img": _prep_image_slab(slab), "cmt": cmt,
                        "vmat": vmat})

    nc = _get_nc(PIX_PER_CORE)
    res = run_bass_kernel_spmd(nc, in_maps, core_ids=list(range(N_CORES)),
                               trace=_trace)
    out = np.empty((NB, H, W), np.float32)
    for i in range(N_CORES):
        n = (i * rows_per_core) // H
        h0 = (i * rows_per_core) % H
        out[n, h0:h0 + rows_per_core, :] = \
            res.results[i]["vel"].reshape(rows_per_core, W)
    return out, res


def kernel(image, cmap, v_i):
    out, _ = _kernel_impl(image, cmap, v_i)
    return out
